# revision 7
# baseline (speedup 1.0000x reference)
"""MoE gate (softmax + top-2) Trainium2 Bass kernel.

Problem: hidden_states [4, 8192, 4096] fp32, weight [16, 4096] fp32.
  logits = x @ W.T -> softmax -> top-2 (values fp32 [32768,2], indices int32 [32768,2])

Sharding: flattened token dim (32768) split across 8 cores (4096 tokens each);
weight replicated.

Strategy (v4):
  Host splits x into exact bf16 hi/lo pairs (x == xh + xl up to ~2^-17 rel) and
  ships them PRE-TRANSPOSED so the contraction dim d lands on SBUF partitions.
  DRAM layout is quarter-contiguous: for each (group, quarter, partition) the
  8 chunks x {hi,lo} x 512 tokens = 16KB are contiguous, so every 2MB
  quarter-load is one descriptor per partition line (max DMA efficiency).
  Quarter-granularity SBUF tiles (9 bufs) let the DMA stream run ~2 groups
  ahead of matmul consumption with fine-grained WAR release.

  logits = xh@wh + xh@wl + xl@wh + xl@wl, every bf16 product exact in fp32.
  The hi and lo weights are packed into one M=48 stationary
  [wh_c | 0 | wl_c] (the zero gap keeps the lo half 32-partition-aligned in
  PSUM, a hardware requirement for the later DVE read), so each x stream
  (xh_c, xl_c) is streamed ONCE against both weight halves: 2 matmuls per
  d-chunk (vs 4 with M=16). Even chunks accumulate at PE column-group 0
  (PSUM rows 0-47), odd chunks at column-group 64 (rows 64-111), giving two
  concurrent moving streams. Final logits.T [16,512] = rows(0:16) +
  rows(32:48) + rows(64:80) + rows(96:112) -- fp32-exact class numerics
  (verified: 0/65536 index mismatches vs fp32 reference).

  Per 512-token group: 64 matmuls accumulate; ACT copy + DVE adds form
  logits.T; PE transposes back to [128,16] per token tile; DVE max/max_index
  gives exact top-2; ACT exp + accum gives softmax denominator. Outputs are
  packed into one [16,1024] tensor per core (rows = (token_tile,
  {v1,v2,i1,i2})); host untangles + casts indices.
"""

import numpy as np
import ml_dtypes

TOK_PER_CORE = 4096
D = 4096
E = 16
N_CORES = 8
GROUP_TOK = 512
N_GROUPS = TOK_PER_CORE // GROUP_TOK  # 8
N_CHUNKS = D // 128  # 32
N_TILES = GROUP_TOK // 128  # 4
QC = N_CHUNKS // 4  # 8 chunks per quarter-load
SEG = 2 * GROUP_TOK  # hi+lo block per chunk (bf16 elems)
WS = 48  # stationary width per chunk: [wh(16) | zeros(16) | wl(16)]

_CACHE = {}


def _build():
    import concourse.bacc as bacc
    import concourse.tile as tile
    from concourse import mybir

    f32 = mybir.dt.float32
    bf16 = mybir.dt.bfloat16
    u32 = mybir.dt.uint32

    nc = bacc.Bacc(None, target_bir_lowering=False, debug=False)
    # xq[g, q, p, (cq s t)] = x_split_s[g*512+t, (q*8+cq)*128+p]
    # -> per (g, q, p) one contiguous 16KB run.
    xq = nc.dram_tensor(
        "xq", [N_GROUPS, 4, 128, QC * SEG], bf16, kind="ExternalInput"
    ).ap()
    # wt[p, c*48 + j] = (j<16 ? wh[j, 128c+p] : j>=32 ? wl[j-32, 128c+p] : 0)
    wt = nc.dram_tensor("wt", [128, N_CHUNKS * WS], bf16, kind="ExternalInput").ap()
    ident = nc.dram_tensor("ident", [128, 128], f32, kind="ExternalInput").ap()
    vt = nc.dram_tensor("vt", [128, N_GROUPS * 16], f32, kind="ExternalOutput").ap()

    with tile.TileContext(nc) as tc:
        with (
            tc.tile_pool(name="const", bufs=1) as cpool,
            tc.tile_pool(name="xload", bufs=9) as xpool,
            tc.tile_pool(name="small", bufs=2) as spool,
            tc.tile_pool(name="acc", bufs=2, space="PSUM") as acc_pool,
            tc.tile_pool(name="mps", bufs=2, space="PSUM") as mps_pool,
        ):
            viacc = cpool.tile([128, N_GROUPS * 16], f32)

            # group-0 x loads first: Q7 descriptor emission is the critical
            # path at kernel start, so x quarters go ahead of the constants
            # (which ride the HWDGE/sync path instead).
            xtiles = {}
            for q in range(4):
                xt = xpool.tile([128, QC * SEG], bf16, tag="xq", name=f"x_0_{q}")
                nc.sync.dma_start(xt[:], xq[0, q])
                xtiles[q] = xt

            wt_sb = cpool.tile([128, N_CHUNKS * WS], bf16)
            nc.scalar.dma_start(wt_sb[:], wt[:])
            id_sb = cpool.tile([128, 128], f32)
            nc.scalar.dma_start(id_sb[:], ident[:])

            def w_ap(c):  # [128, 48] stationary: cols 0-15 wh_c, 32-47 wl_c
                return wt_sb[:, c * WS : (c + 1) * WS]

            for g in range(N_GROUPS):
                # 1. this group's token quarters (group 0 preloaded above)
                if g > 0:
                    eng = nc.sync if g == 1 else nc.gpsimd
                    xtiles = {}
                    for q in range(4):
                        xt = xpool.tile(
                            [128, QC * SEG], bf16, tag="xq", name=f"x_{g}_{q}"
                        )
                        if g == N_GROUPS - 1:
                            # last group in half-quarter pieces: less matmul
                            # work left dangling after the final byte lands
                            half = QC * SEG // 2
                            hsrc = xq[g, q].rearrange("p (h r) -> p h r", h=2)
                            nc.gpsimd.dma_start(xt[:, 0:half], hsrc[:, 0])
                            nc.gpsimd.dma_start(xt[:, half:], hsrc[:, 1])
                        else:
                            eng.dma_start(xt[:], xq[g, q])
                        xtiles[q] = xt

                def xk(c, s):  # [128, 512] moving slice
                    cq = c % QC
                    return xtiles[c // QC][
                        :, (cq * 2 + s) * GROUP_TOK : (cq * 2 + s + 1) * GROUP_TOK
                    ]

                # 2. 64 matmuls accumulate into one PSUM [112, 512] bank:
                # even chunks -> PE col-group 0 (rows 0-47), odd chunks ->
                # col-group 64 (rows 64-111): two concurrent moving streams.
                mp = acc_pool.tile([112, GROUP_TOK], f32, tag="mp", name=f"mp_{g}")
                for c in range(N_CHUNKS):
                    off = 64 * (c % 2)
                    for s in (0, 1):
                        nc.tensor.matmul(
                            mp[off : off + 48, :],
                            w_ap(c),
                            xk(c, s),
                            start=(c < 2 and s == 0),
                            stop=(c >= N_CHUNKS - 2 and s == 1),
                            tile_position=(0, off),
                        )

                # 3. logits.T [16, 512] = rows(0:16)+rows(32:48)+rows(64:80)+rows(96:112)
                # (tensor_tensor may read at most one PSUM input)
                s0 = spool.tile([16, GROUP_TOK], f32, tag="s0")
                nc.scalar.copy(s0[:], mp[0:16, :])
                s1 = spool.tile([16, GROUP_TOK], f32, tag="s1")
                nc.vector.tensor_add(s1[:], s0[:], mp[32:48, :])
                s2 = spool.tile([16, GROUP_TOK], f32, tag="s2")
                nc.vector.tensor_add(s2[:], s1[:], mp[64:80, :])
                lg_sb = spool.tile([16, GROUP_TOK], f32, tag="lgsb")
                nc.vector.tensor_add(lg_sb[:], s2[:], mp[96:112, :])

                # 4. transpose logits back: [16,128] -> [128,16] per token tile
                lgt_ps = mps_pool.tile([128, N_TILES * E], f32, tag="lgt")
                for tt in range(N_TILES):
                    nc.tensor.transpose(
                        lgt_ps[:, tt * E : (tt + 1) * E],
                        lg_sb[:, tt * 128 : (tt + 1) * 128],
                        id_sb[0:16, 0:16],
                    )
                lgt_sb = spool.tile([128, N_TILES * E], f32, tag="lgtsb")
                nc.vector.tensor_copy(lgt_sb[:], lgt_ps[:])

                # 5. top-2 + softmax per token tile
                vi = viacc[:, g * 16 : (g + 1) * 16]
                for tt in range(N_TILES):
                    lt = lgt_sb[:, tt * E : (tt + 1) * E]
                    mx = spool.tile([128, 8], f32, tag=f"mx{tt}")
                    nc.vector.max(mx[:], lt)
                    ix = spool.tile([128, 8], u32, tag=f"ix{tt}")
                    nc.vector.max_index(ix[:], mx[:], lt)
                    ex = spool.tile([128, E], f32, tag=f"ex{tt}")
                    s = spool.tile([128, 1], f32, tag=f"s{tt}")
                    nc.scalar.activation(
                        ex[:], lt, mybir.ActivationFunctionType.Exp, accum_out=s[:]
                    )
                    em = spool.tile([128, 2], f32, tag=f"em{tt}")
                    nc.scalar.activation(
                        em[:], mx[:, 0:2], mybir.ActivationFunctionType.Exp
                    )
                    rs = spool.tile([128, 1], f32, tag=f"rs{tt}")
                    nc.vector.reciprocal(rs[:], s[:])
                    nc.vector.tensor_scalar_mul(
                        vi[:, tt * 4 : tt * 4 + 2], em[:], rs[:]
                    )
                    nc.vector.tensor_copy(vi[:, tt * 4 + 2 : tt * 4 + 4], ix[:, 0:2])

                # store this group's packed outputs right away (GpSimd is
                # otherwise idle; keeps the kernel tail to one 8KB store)
                nc.scalar.dma_start(vt[:, g * 16 : (g + 1) * 16], vi)

    nc.compile()
    return nc


def _get_nc():
    if "nc" not in _CACHE:
        _CACHE["nc"] = _build()
    return _CACHE["nc"]


def _prep_inputs(hidden_states, weight):
    bf = ml_dtypes.bfloat16
    x = np.ascontiguousarray(hidden_states, dtype=np.float32).reshape(-1, D)
    w = np.ascontiguousarray(weight, dtype=np.float32)

    xh = x.astype(bf)
    xl = (x - xh.astype(np.float32)).astype(bf)
    wh = w.astype(bf)
    wl = (w - wh.astype(np.float32)).astype(bf)

    # wt[p, c*48 + j] = (j<16 ? wh[j, 128c+p] : j>=32 ? wl[j-32, 128c+p] : 0)
    wt3 = np.zeros((128, N_CHUNKS, WS), dtype=bf)
    wt3[:, :, 0:E] = wh.reshape(E, N_CHUNKS, 128).transpose(2, 1, 0)
    wt3[:, :, 2 * E : 3 * E] = wl.reshape(E, N_CHUNKS, 128).transpose(2, 1, 0)
    wt = np.ascontiguousarray(wt3.reshape(128, N_CHUNKS * WS))
    ident = np.eye(128, dtype=np.float32)

    in_maps = []
    for core in range(N_CORES):
        sl = slice(core * TOK_PER_CORE, (core + 1) * TOK_PER_CORE)
        # xq[g, q, p, cq, s, t] = x_split_s[core_tok0 + g*512 + t, (q*8+cq)*128+p]
        xs_all = np.stack([xh[sl], xl[sl]], axis=0)  # [s, tok, d]
        a = xs_all.reshape(2, N_GROUPS, GROUP_TOK, N_CHUNKS, 128)  # [s, g, t, c, p]
        a = a.transpose(1, 3, 4, 0, 2)  # [g, c, p, s, t]
        a = a.reshape(N_GROUPS, 4, QC, 128, 2, GROUP_TOK)  # [g, q, cq, p, s, t]
        a = a.transpose(0, 1, 3, 2, 4, 5)  # [g, q, p, cq, s, t]
        xqa = np.ascontiguousarray(a.reshape(N_GROUPS, 4, 128, QC * SEG))
        in_maps.append({"xq": xqa, "wt": wt, "ident": ident})
    return in_maps


def _postprocess(results):
    vals_all = []
    idx_all = []
    for core in range(N_CORES):
        arr = results[core]["vt"]  # [128, 8*16]
        # arr[tl, g*16 + tt*4 + k] -> token g*512+tt*128+tl
        a = arr.reshape(128, N_GROUPS, N_TILES, 4)  # [tl, g, tt, k]
        a = a.transpose(1, 2, 0, 3).reshape(TOK_PER_CORE, 4)  # [(g,tt,tl), k]
        vals_all.append(a[:, 0:2].astype(np.float32))
        idx_all.append(np.rint(a[:, 2:4]).astype(np.int32))
    values = np.concatenate(vals_all, axis=0)
    indices = np.concatenate(idx_all, axis=0)
    return values, indices


def kernel(hidden_states, weight):
    from concourse.bass_utils import run_bass_kernel_spmd

    nc = _get_nc()
    in_maps = _prep_inputs(hidden_states, weight)
    res = run_bass_kernel_spmd(nc, in_maps, list(range(N_CORES)))
    return _postprocess(res.results)


def run_traced(hidden_states, weight, **kwargs):
    """For test.py: same as kernel() but returns (outputs, BassKernelResults)."""
    from concourse.bass_utils import run_bass_kernel_spmd

    nc = _get_nc()
    in_maps = _prep_inputs(hidden_states, weight)
    res = run_bass_kernel_spmd(nc, in_maps, list(range(N_CORES)), **kwargs)
    return _postprocess(res.results), res


# revision 8
# speedup vs baseline: 1.0327x; 1.0327x over previous
"""MoE gate (softmax + top-2) Trainium2 Bass kernel.

Problem: hidden_states [4, 8192, 4096] fp32, weight [16, 4096] fp32.
  logits = x @ W.T -> softmax -> top-2 (values fp32 [32768,2], indices int32 [32768,2])

Sharding: flattened token dim (32768) split across 8 cores (4096 tokens each);
weight replicated.

Strategy (v4):
  Host splits x into exact bf16 hi/lo pairs (x == xh + xl up to ~2^-17 rel) and
  ships them PRE-TRANSPOSED so the contraction dim d lands on SBUF partitions.
  DRAM layout is quarter-contiguous: for each (group, quarter, partition) the
  8 chunks x {hi,lo} x 512 tokens = 16KB are contiguous, so every 2MB
  quarter-load is one descriptor per partition line (max DMA efficiency).
  Quarter-granularity SBUF tiles (9 bufs) let the DMA stream run ~2 groups
  ahead of matmul consumption with fine-grained WAR release.

  logits = xh@wh + xh@wl + xl@wh + xl@wl, every bf16 product exact in fp32.
  The hi and lo weights are packed into one M=48 stationary
  [wh_c | 0 | wl_c] (the zero gap keeps the lo half 32-partition-aligned in
  PSUM, a hardware requirement for the later DVE read), so each x stream
  (xh_c, xl_c) is streamed ONCE against both weight halves: 2 matmuls per
  d-chunk (vs 4 with M=16). Even chunks accumulate at PE column-group 0
  (PSUM rows 0-47), odd chunks at column-group 64 (rows 64-111), giving two
  concurrent moving streams. Final logits.T [16,512] = rows(0:16) +
  rows(32:48) + rows(64:80) + rows(96:112) -- fp32-exact class numerics
  (verified: 0/65536 index mismatches vs fp32 reference).

  Per 512-token group: 64 matmuls accumulate; ACT copy + DVE adds form
  logits.T; PE transposes back to [128,16] per token tile; DVE max/max_index
  gives exact top-2; ACT exp + accum gives softmax denominator. Outputs are
  packed into one [16,1024] tensor per core (rows = (token_tile,
  {v1,v2,i1,i2})); host untangles + casts indices.
"""

import numpy as np
import ml_dtypes

TOK_PER_CORE = 4096
D = 4096
E = 16
N_CORES = 8
GROUP_TOK = 512
N_GROUPS = TOK_PER_CORE // GROUP_TOK  # 8
N_CHUNKS = D // 128  # 32
N_TILES = GROUP_TOK // 128  # 4
QC = N_CHUNKS // 4  # 8 chunks per quarter-load
SEG = 2 * GROUP_TOK  # hi+lo block per chunk (bf16 elems)
WS = 48  # stationary width per chunk: [wh(16) | zeros(16) | wl(16)]

_CACHE = {}


def _build():
    import concourse.bacc as bacc
    import concourse.tile as tile
    from concourse import mybir

    f32 = mybir.dt.float32
    bf16 = mybir.dt.bfloat16
    u32 = mybir.dt.uint32

    nc = bacc.Bacc(None, target_bir_lowering=False, debug=False)
    # xq[g, q, p, (cq s t)] = x_split_s[g*512+t, (q*8+cq)*128+p]
    # -> per (g, q, p) one contiguous 16KB run.
    xq = nc.dram_tensor(
        "xq", [N_GROUPS, 4, 128, QC * SEG], bf16, kind="ExternalInput"
    ).ap()
    # wt[p, c*48 + j] = (j<16 ? wh[j, 128c+p] : j>=32 ? wl[j-32, 128c+p] : 0)
    wt = nc.dram_tensor("wt", [128, N_CHUNKS * WS], bf16, kind="ExternalInput").ap()
    ident = nc.dram_tensor("ident", [128, 128], f32, kind="ExternalInput").ap()
    vt = nc.dram_tensor("vt", [128, N_GROUPS * 16], f32, kind="ExternalOutput").ap()

    with tile.TileContext(nc) as tc:
        with (
            tc.tile_pool(name="const", bufs=1) as cpool,
            tc.tile_pool(name="xload", bufs=9) as xpool,
            tc.tile_pool(name="small", bufs=2) as spool,
            tc.tile_pool(name="acc", bufs=2, space="PSUM") as acc_pool,
            tc.tile_pool(name="mps", bufs=2, space="PSUM") as mps_pool,
        ):
            viacc = cpool.tile([128, N_GROUPS * 16], f32)

            # group-0 x loads first: Q7 descriptor emission is the critical
            # path at kernel start, so x quarters go ahead of the constants
            # (which ride the HWDGE/sync path instead).
            xtiles = {}
            for q in range(4):
                xt = xpool.tile([128, QC * SEG], bf16, tag="xq", name=f"x_0_{q}")
                nc.gpsimd.dma_start(xt[:], xq[0, q])
                xtiles[q] = xt

            wt_sb = cpool.tile([128, N_CHUNKS * WS], bf16)
            nc.sync.dma_start(wt_sb[:], wt[:])
            id_sb = cpool.tile([128, 128], f32)
            nc.sync.dma_start(id_sb[:], ident[:])

            def w_ap(c):  # [128, 48] stationary: cols 0-15 wh_c, 32-47 wl_c
                return wt_sb[:, c * WS : (c + 1) * WS]

            for g in range(N_GROUPS):
                # 1. this group's token quarters (group 0 preloaded above)
                if g > 0:
                    xtiles = {}
                    for q in range(4):
                        xt = xpool.tile(
                            [128, QC * SEG], bf16, tag="xq", name=f"x_{g}_{q}"
                        )
                        if g == N_GROUPS - 1:
                            # last group in half-quarter pieces: less matmul
                            # work left dangling after the final byte lands
                            half = QC * SEG // 2
                            hsrc = xq[g, q].rearrange("p (h r) -> p h r", h=2)
                            nc.gpsimd.dma_start(xt[:, 0:half], hsrc[:, 0])
                            nc.gpsimd.dma_start(xt[:, half:], hsrc[:, 1])
                        else:
                            nc.gpsimd.dma_start(xt[:], xq[g, q])
                        xtiles[q] = xt

                def xk(c, s):  # [128, 512] moving slice
                    cq = c % QC
                    return xtiles[c // QC][
                        :, (cq * 2 + s) * GROUP_TOK : (cq * 2 + s + 1) * GROUP_TOK
                    ]

                # 2. 64 matmuls accumulate into one PSUM [112, 512] bank:
                # even chunks -> PE col-group 0 (rows 0-47), odd chunks ->
                # col-group 64 (rows 64-111): two concurrent moving streams.
                mp = acc_pool.tile([112, GROUP_TOK], f32, tag="mp", name=f"mp_{g}")
                for c in range(N_CHUNKS):
                    off = 64 * (c % 2)
                    for s in (0, 1):
                        nc.tensor.matmul(
                            mp[off : off + 48, :],
                            w_ap(c),
                            xk(c, s),
                            start=(c < 2 and s == 0),
                            stop=(c >= N_CHUNKS - 2 and s == 1),
                            tile_position=(0, off),
                        )

                # 3. logits.T [16, 512] = rows(0:16)+rows(32:48)+rows(64:80)+rows(96:112)
                # (tensor_tensor may read at most one PSUM input)
                s0 = spool.tile([16, GROUP_TOK], f32, tag="s0")
                nc.scalar.copy(s0[:], mp[0:16, :])
                s1 = spool.tile([16, GROUP_TOK], f32, tag="s1")
                nc.vector.tensor_add(s1[:], s0[:], mp[32:48, :])
                s2 = spool.tile([16, GROUP_TOK], f32, tag="s2")
                nc.vector.tensor_add(s2[:], s1[:], mp[64:80, :])
                lg_sb = spool.tile([16, GROUP_TOK], f32, tag="lgsb")
                nc.vector.tensor_add(lg_sb[:], s2[:], mp[96:112, :])

                # 4. transpose logits back: [16,128] -> [128,16] per token tile
                lgt_ps = mps_pool.tile([128, N_TILES * E], f32, tag="lgt")
                for tt in range(N_TILES):
                    nc.tensor.transpose(
                        lgt_ps[:, tt * E : (tt + 1) * E],
                        lg_sb[:, tt * 128 : (tt + 1) * 128],
                        id_sb[0:16, 0:16],
                    )
                lgt_sb = spool.tile([128, N_TILES * E], f32, tag="lgtsb")
                nc.vector.tensor_copy(lgt_sb[:], lgt_ps[:])

                # 5. top-2 + softmax per token tile
                vi = viacc[:, g * 16 : (g + 1) * 16]
                for tt in range(N_TILES):
                    lt = lgt_sb[:, tt * E : (tt + 1) * E]
                    mx = spool.tile([128, 8], f32, tag=f"mx{tt}")
                    nc.vector.max(mx[:], lt)
                    ix = spool.tile([128, 8], u32, tag=f"ix{tt}")
                    nc.vector.max_index(ix[:], mx[:], lt)
                    ex = spool.tile([128, E], f32, tag=f"ex{tt}")
                    s = spool.tile([128, 1], f32, tag=f"s{tt}")
                    nc.scalar.activation(
                        ex[:], lt, mybir.ActivationFunctionType.Exp, accum_out=s[:]
                    )
                    em = spool.tile([128, 2], f32, tag=f"em{tt}")
                    nc.scalar.activation(
                        em[:], mx[:, 0:2], mybir.ActivationFunctionType.Exp
                    )
                    rs = spool.tile([128, 1], f32, tag=f"rs{tt}")
                    nc.vector.reciprocal(rs[:], s[:])
                    nc.vector.tensor_scalar_mul(
                        vi[:, tt * 4 : tt * 4 + 2], em[:], rs[:]
                    )
                    nc.vector.tensor_copy(vi[:, tt * 4 + 2 : tt * 4 + 4], ix[:, 0:2])

                # store this group's packed outputs right away (GpSimd is
                # otherwise idle; keeps the kernel tail to one 8KB store)
                nc.scalar.dma_start(vt[:, g * 16 : (g + 1) * 16], vi)

    nc.compile()
    return nc


def _get_nc():
    if "nc" not in _CACHE:
        _CACHE["nc"] = _build()
    return _CACHE["nc"]


def _prep_inputs(hidden_states, weight):
    bf = ml_dtypes.bfloat16
    x = np.ascontiguousarray(hidden_states, dtype=np.float32).reshape(-1, D)
    w = np.ascontiguousarray(weight, dtype=np.float32)

    xh = x.astype(bf)
    xl = (x - xh.astype(np.float32)).astype(bf)
    wh = w.astype(bf)
    wl = (w - wh.astype(np.float32)).astype(bf)

    # wt[p, c*48 + j] = (j<16 ? wh[j, 128c+p] : j>=32 ? wl[j-32, 128c+p] : 0)
    wt3 = np.zeros((128, N_CHUNKS, WS), dtype=bf)
    wt3[:, :, 0:E] = wh.reshape(E, N_CHUNKS, 128).transpose(2, 1, 0)
    wt3[:, :, 2 * E : 3 * E] = wl.reshape(E, N_CHUNKS, 128).transpose(2, 1, 0)
    wt = np.ascontiguousarray(wt3.reshape(128, N_CHUNKS * WS))
    ident = np.eye(128, dtype=np.float32)

    in_maps = []
    for core in range(N_CORES):
        sl = slice(core * TOK_PER_CORE, (core + 1) * TOK_PER_CORE)
        # xq[g, q, p, cq, s, t] = x_split_s[core_tok0 + g*512 + t, (q*8+cq)*128+p]
        xs_all = np.stack([xh[sl], xl[sl]], axis=0)  # [s, tok, d]
        a = xs_all.reshape(2, N_GROUPS, GROUP_TOK, N_CHUNKS, 128)  # [s, g, t, c, p]
        a = a.transpose(1, 3, 4, 0, 2)  # [g, c, p, s, t]
        a = a.reshape(N_GROUPS, 4, QC, 128, 2, GROUP_TOK)  # [g, q, cq, p, s, t]
        a = a.transpose(0, 1, 3, 2, 4, 5)  # [g, q, p, cq, s, t]
        xqa = np.ascontiguousarray(a.reshape(N_GROUPS, 4, 128, QC * SEG))
        in_maps.append({"xq": xqa, "wt": wt, "ident": ident})
    return in_maps


def _postprocess(results):
    vals_all = []
    idx_all = []
    for core in range(N_CORES):
        arr = results[core]["vt"]  # [128, 8*16]
        # arr[tl, g*16 + tt*4 + k] -> token g*512+tt*128+tl
        a = arr.reshape(128, N_GROUPS, N_TILES, 4)  # [tl, g, tt, k]
        a = a.transpose(1, 2, 0, 3).reshape(TOK_PER_CORE, 4)  # [(g,tt,tl), k]
        vals_all.append(a[:, 0:2].astype(np.float32))
        idx_all.append(np.rint(a[:, 2:4]).astype(np.int32))
    values = np.concatenate(vals_all, axis=0)
    indices = np.concatenate(idx_all, axis=0)
    return values, indices


def kernel(hidden_states, weight):
    from concourse.bass_utils import run_bass_kernel_spmd

    nc = _get_nc()
    in_maps = _prep_inputs(hidden_states, weight)
    res = run_bass_kernel_spmd(nc, in_maps, list(range(N_CORES)))
    return _postprocess(res.results)


def run_traced(hidden_states, weight, **kwargs):
    """For test.py: same as kernel() but returns (outputs, BassKernelResults)."""
    from concourse.bass_utils import run_bass_kernel_spmd

    nc = _get_nc()
    in_maps = _prep_inputs(hidden_states, weight)
    res = run_bass_kernel_spmd(nc, in_maps, list(range(N_CORES)), **kwargs)
    return _postprocess(res.results), res


# revision 9
# speedup vs baseline: 1.2035x; 1.1654x over previous
"""MoE gate (softmax + top-2) Trainium2 Bass kernel.

Problem: hidden_states [4, 8192, 4096] fp32, weight [16, 4096] fp32.
  logits = x @ W.T -> softmax -> top-2 (values fp32 [32768,2], indices int32 [32768,2])

Sharding: flattened token dim (32768) split across 8 cores (4096 tokens each);
weight replicated.

Strategy (v4):
  Host splits x into exact bf16 hi/lo pairs (x == xh + xl up to ~2^-17 rel) and
  ships them PRE-TRANSPOSED so the contraction dim d lands on SBUF partitions.
  DRAM layout is quarter-contiguous: for each (group, quarter, partition) the
  8 chunks x {hi,lo} x 512 tokens = 16KB are contiguous, so every 2MB
  quarter-load is one descriptor per partition line (max DMA efficiency).
  Quarter-granularity SBUF tiles (9 bufs) let the DMA stream run ~2 groups
  ahead of matmul consumption with fine-grained WAR release.

  logits = xh@wh + xh@wl + xl@wh + xl@wl, every bf16 product exact in fp32.
  The hi and lo weights are packed into one M=48 stationary
  [wh_c | 0 | wl_c] (the zero gap keeps the lo half 32-partition-aligned in
  PSUM, a hardware requirement for the later DVE read), so each x stream
  (xh_c, xl_c) is streamed ONCE against both weight halves: 2 matmuls per
  d-chunk (vs 4 with M=16). Even chunks accumulate at PE column-group 0
  (PSUM rows 0-47), odd chunks at column-group 64 (rows 64-111), giving two
  concurrent moving streams. Final logits.T [16,512] = rows(0:16) +
  rows(32:48) + rows(64:80) + rows(96:112) -- fp32-exact class numerics
  (verified: 0/65536 index mismatches vs fp32 reference).

  Per 512-token group: 64 matmuls accumulate; ACT copy + DVE adds form
  logits.T; PE transposes back to [128,16] per token tile; DVE max/max_index
  gives exact top-2; ACT exp + accum gives softmax denominator. Outputs are
  packed into one [16,1024] tensor per core (rows = (token_tile,
  {v1,v2,i1,i2})); host untangles + casts indices.
"""

import numpy as np
import ml_dtypes

TOK_PER_CORE = 4096
D = 4096
E = 16
N_CORES = 8
GROUP_TOK = 512
N_GROUPS = TOK_PER_CORE // GROUP_TOK  # 8
N_CHUNKS = D // 128  # 32
N_TILES = GROUP_TOK // 128  # 4
QC = N_CHUNKS // 4  # 8 chunks per quarter-load
SEG = 2 * GROUP_TOK  # hi+lo block per chunk (bf16 elems)
WS = 48  # stationary width per chunk: [wh(16) | zeros(16) | wl(16)]

_CACHE = {}


def _build():
    import concourse.bacc as bacc
    import concourse.tile as tile
    from concourse import mybir

    f32 = mybir.dt.float32
    bf16 = mybir.dt.bfloat16
    u32 = mybir.dt.uint32

    nc = bacc.Bacc(None, target_bir_lowering=False, debug=False)
    # xq[g, q, p, (cq s t)] = x_split_s[g*512+t, (q*8+cq)*128+p]
    # -> per (g, q, p) one contiguous 16KB run.
    xq = nc.dram_tensor(
        "xq", [N_GROUPS, 4, 128, QC * SEG], bf16, kind="ExternalInput"
    ).ap()
    # wt[p, c*48 + j] = (j<16 ? wh[j, 128c+p] : j>=32 ? wl[j-32, 128c+p] : 0)
    wt = nc.dram_tensor("wt", [128, N_CHUNKS * WS], bf16, kind="ExternalInput").ap()
    ident = nc.dram_tensor("ident", [128, 128], f32, kind="ExternalInput").ap()
    vt = nc.dram_tensor("vt", [128, N_GROUPS * 16], f32, kind="ExternalOutput").ap()

    with tile.TileContext(nc) as tc:
        with (
            tc.tile_pool(name="const", bufs=1) as cpool,
            tc.tile_pool(name="xload", bufs=9) as xpool,
            tc.tile_pool(name="small", bufs=2) as spool,
            tc.tile_pool(name="acc", bufs=2, space="PSUM") as acc_pool,
            tc.tile_pool(name="mps", bufs=2, space="PSUM") as mps_pool,
        ):
            viacc = cpool.tile([128, N_GROUPS * 16], f32)

            # group-0 x loads first: Q7 descriptor emission is the critical
            # path at kernel start, so x quarters go ahead of the constants
            # (which ride the HWDGE/sync path instead).
            xtiles = {}
            for q in range(4):
                xt = xpool.tile([128, QC * SEG], bf16, tag="xq", name=f"x_0_{q}")
                nc.gpsimd.dma_start(xt[:], xq[0, q])
                xtiles[q] = xt

            wt_sb = cpool.tile([128, N_CHUNKS * WS], bf16)
            nc.sync.dma_start(wt_sb[:], wt[:])
            id_sb = cpool.tile([128, 128], f32)
            nc.sync.dma_start(id_sb[:], ident[:])

            def w_ap(c):  # [128, 48] stationary: cols 0-15 wh_c, 32-47 wl_c
                return wt_sb[:, c * WS : (c + 1) * WS]

            for g in range(N_GROUPS):
                # 1. this group's token quarters (group 0 preloaded above)
                if g > 0:
                    xtiles = {}
                    for q in range(4):
                        xt = xpool.tile(
                            [128, QC * SEG], bf16, tag="xq", name=f"x_{g}_{q}"
                        )
                        if g == N_GROUPS - 1:
                            # last group in half-quarter pieces: less matmul
                            # work left dangling after the final byte lands
                            half = QC * SEG // 2
                            hsrc = xq[g, q].rearrange("p (h r) -> p h r", h=2)
                            nc.gpsimd.dma_start(xt[:, 0:half], hsrc[:, 0])
                            nc.gpsimd.dma_start(xt[:, half:], hsrc[:, 1])
                        else:
                            nc.gpsimd.dma_start(xt[:], xq[g, q])
                        xtiles[q] = xt

                def xk(c, s):  # [128, 512] moving slice
                    cq = c % QC
                    return xtiles[c // QC][
                        :, (cq * 2 + s) * GROUP_TOK : (cq * 2 + s + 1) * GROUP_TOK
                    ]

                # 2. 64 matmuls accumulate into one PSUM [112, 512] bank:
                # even chunks -> PE col-group 0 (rows 0-47), odd chunks ->
                # col-group 64 (rows 64-111): two concurrent moving streams.
                mp = acc_pool.tile([112, GROUP_TOK], f32, tag="mp", name=f"mp_{g}")
                for c in range(N_CHUNKS):
                    off = 64 * (c % 2)
                    for s in (0, 1):
                        nc.tensor.matmul(
                            mp[off : off + 48, :],
                            w_ap(c),
                            xk(c, s),
                            start=(c < 2 and s == 0),
                            stop=(c >= N_CHUNKS - 2 and s == 1),
                            tile_position=(0, off),
                        )

                # 3. logits.T [16, 512] = rows(0:16)+rows(32:48)+rows(64:80)+rows(96:112)
                # (tensor_tensor may read at most one PSUM input)
                s0 = spool.tile([16, GROUP_TOK], f32, tag="s0")
                nc.scalar.copy(s0[:], mp[0:16, :])
                s1 = spool.tile([16, GROUP_TOK], f32, tag="s1")
                nc.vector.tensor_add(s1[:], s0[:], mp[32:48, :])
                s2 = spool.tile([16, GROUP_TOK], f32, tag="s2")
                nc.vector.tensor_add(s2[:], s1[:], mp[64:80, :])
                lg_sb = spool.tile([16, GROUP_TOK], f32, tag="lgsb")
                nc.vector.tensor_add(lg_sb[:], s2[:], mp[96:112, :])

                # 4. transpose logits back: [16,128] -> [128,16] per token tile
                lgt_ps = mps_pool.tile([128, N_TILES * E], f32, tag="lgt")
                for tt in range(N_TILES):
                    nc.tensor.transpose(
                        lgt_ps[:, tt * E : (tt + 1) * E],
                        lg_sb[:, tt * 128 : (tt + 1) * 128],
                        id_sb[0:16, 0:16],
                    )
                lgt_sb = spool.tile([128, N_TILES * E], f32, tag="lgtsb")
                nc.vector.tensor_copy(lgt_sb[:], lgt_ps[:])

                # 5. top-2 + softmax per token tile
                vi = viacc[:, g * 16 : (g + 1) * 16]
                for tt in range(N_TILES):
                    lt = lgt_sb[:, tt * E : (tt + 1) * E]
                    mx = spool.tile([128, 8], f32, tag=f"mx{tt}")
                    nc.vector.max(mx[:], lt)
                    ix = spool.tile([128, 8], u32, tag=f"ix{tt}")
                    nc.vector.max_index(ix[:], mx[:], lt)
                    ex = spool.tile([128, E], f32, tag=f"ex{tt}")
                    s = spool.tile([128, 1], f32, tag=f"s{tt}")
                    nc.scalar.activation(
                        ex[:], lt, mybir.ActivationFunctionType.Exp, accum_out=s[:]
                    )
                    em = spool.tile([128, 2], f32, tag=f"em{tt}")
                    nc.scalar.activation(
                        em[:], mx[:, 0:2], mybir.ActivationFunctionType.Exp
                    )
                    rs = spool.tile([128, 1], f32, tag=f"rs{tt}")
                    nc.vector.reciprocal(rs[:], s[:])
                    nc.vector.tensor_scalar_mul(
                        vi[:, tt * 4 : tt * 4 + 2], em[:], rs[:]
                    )
                    nc.vector.tensor_copy(vi[:, tt * 4 + 2 : tt * 4 + 4], ix[:, 0:2])

            # single store at the end: a per-group store would park its
            # engine's strict FIFO on the vi-ready semaphore (measured 40us+
            # head-of-line stalls on ACT/Q7), starving everything behind it
            nc.gpsimd.dma_start(vt[:], viacc[:])

    nc.compile()
    return nc


def _get_nc():
    if "nc" not in _CACHE:
        _CACHE["nc"] = _build()
    return _CACHE["nc"]


def _prep_inputs(hidden_states, weight):
    bf = ml_dtypes.bfloat16
    x = np.ascontiguousarray(hidden_states, dtype=np.float32).reshape(-1, D)
    w = np.ascontiguousarray(weight, dtype=np.float32)

    xh = x.astype(bf)
    xl = (x - xh.astype(np.float32)).astype(bf)
    wh = w.astype(bf)
    wl = (w - wh.astype(np.float32)).astype(bf)

    # wt[p, c*48 + j] = (j<16 ? wh[j, 128c+p] : j>=32 ? wl[j-32, 128c+p] : 0)
    wt3 = np.zeros((128, N_CHUNKS, WS), dtype=bf)
    wt3[:, :, 0:E] = wh.reshape(E, N_CHUNKS, 128).transpose(2, 1, 0)
    wt3[:, :, 2 * E : 3 * E] = wl.reshape(E, N_CHUNKS, 128).transpose(2, 1, 0)
    wt = np.ascontiguousarray(wt3.reshape(128, N_CHUNKS * WS))
    ident = np.eye(128, dtype=np.float32)

    in_maps = []
    for core in range(N_CORES):
        sl = slice(core * TOK_PER_CORE, (core + 1) * TOK_PER_CORE)
        # xq[g, q, p, cq, s, t] = x_split_s[core_tok0 + g*512 + t, (q*8+cq)*128+p]
        xs_all = np.stack([xh[sl], xl[sl]], axis=0)  # [s, tok, d]
        a = xs_all.reshape(2, N_GROUPS, GROUP_TOK, N_CHUNKS, 128)  # [s, g, t, c, p]
        a = a.transpose(1, 3, 4, 0, 2)  # [g, c, p, s, t]
        a = a.reshape(N_GROUPS, 4, QC, 128, 2, GROUP_TOK)  # [g, q, cq, p, s, t]
        a = a.transpose(0, 1, 3, 2, 4, 5)  # [g, q, p, cq, s, t]
        xqa = np.ascontiguousarray(a.reshape(N_GROUPS, 4, 128, QC * SEG))
        in_maps.append({"xq": xqa, "wt": wt, "ident": ident})
    return in_maps


def _postprocess(results):
    vals_all = []
    idx_all = []
    for core in range(N_CORES):
        arr = results[core]["vt"]  # [128, 8*16]
        # arr[tl, g*16 + tt*4 + k] -> token g*512+tt*128+tl
        a = arr.reshape(128, N_GROUPS, N_TILES, 4)  # [tl, g, tt, k]
        a = a.transpose(1, 2, 0, 3).reshape(TOK_PER_CORE, 4)  # [(g,tt,tl), k]
        vals_all.append(a[:, 0:2].astype(np.float32))
        idx_all.append(np.rint(a[:, 2:4]).astype(np.int32))
    values = np.concatenate(vals_all, axis=0)
    indices = np.concatenate(idx_all, axis=0)
    return values, indices


def kernel(hidden_states, weight):
    from concourse.bass_utils import run_bass_kernel_spmd

    nc = _get_nc()
    in_maps = _prep_inputs(hidden_states, weight)
    res = run_bass_kernel_spmd(nc, in_maps, list(range(N_CORES)))
    return _postprocess(res.results)


def run_traced(hidden_states, weight, **kwargs):
    """For test.py: same as kernel() but returns (outputs, BassKernelResults)."""
    from concourse.bass_utils import run_bass_kernel_spmd

    nc = _get_nc()
    in_maps = _prep_inputs(hidden_states, weight)
    res = run_bass_kernel_spmd(nc, in_maps, list(range(N_CORES)), **kwargs)
    return _postprocess(res.results), res
